# revision 9
# baseline (speedup 1.0000x reference)
"""Distributed single-head attention block for trn2 (8 NeuronCores), v5.

reference:
    q = x @ Wq.T + bq ; k = x @ Wk.T + bk ; v = x @ Wv.T + bv
    out = x + softmax(q @ k.T / sqrt(D)) @ v       x: [4, 2048, 1024]

Sharding: 8 cores = 4 batches x 2 halves. Core c owns batch c//2 and
rows [h*1024, (h+1)*1024) with h = c%2 — both as queries and as keys.
Each core projects Q, K and V only for its OWN half (perfectly
balanced: 3 x 1024^3 MACs of projection per core vs 4 x 1024^3 in the
K-replicated variant) and the pair exchanges K/V halves with two
pairwise AllGathers (2MB each), landing in natural batch order.

Scheduling model (measured): the tile scheduler places every
instruction as early in its engine's in-order queue as the EMISSION
positions of its dependencies allow — emission order is only a
tiebreak. A dma_start costs ~0.7us of issue time on the triggering
engine's queue, and a blocked DMA at the gpsimd queue head stalls the
collective handshake machinery (the CC stream runs on the gpsimd
cores), so:
    sync   : input loads, then the 4 big slot readbacks (K before V)
    scalar : bq load + K/V stage-outs (early), exp activations, output
             stores
    vector : psum->SBUF casts and all other DVE math only
    gpsimd : ONLY the two collective triggers
Consecutive collectives serialize on the single CC stream including
~15us of per-op trigger latency: K (needed at ~93us) goes first, V
(needed at ~160us) second. The qT loop is ec-outer so the first
hoisted score chain keeps 15/16 of the qT chains as PE cover while the
K AllGather lands. Scores and attention share ONE psum ring (same
tag), which both gives scores 4-deep exp backpressure slack and pins
hoisted attention chains to ~pass-1 end, after the V readback.

Device-side layouts (host pre-transposes + bf16-casts so the
contraction dim always lands on SBUF partitions):
    xqT  [D, SQ]            bf16  x[b, half].T   -> Q/K/V projections
    wvT  [D, D]             bf16  Wv.T
    wkE  [EC, 128, DC, 128] bf16  Wk.T e-chunk-major (wkE[ec,p,dc,j]
          = Wk.T[dc*128+p, ec*128+j]) so the first kT chain only needs
          xqT + one 256KB e-chunk: first real matmul at ~8us.
    wqE  same for Wq.T
The device returns softmax(qk/sqrt(D))@v per owned half; the host adds
the residual x + bv (attention weights sum to 1, so the V bias is a
plain output offset; the K bias cancels in softmax; only the Q bias is
applied on-device). Projections emit qT/kT [e, s] (scores contraction
over e) and v [s, e] (attn contraction over keys). Softmax rows live
on partitions: exp on ScalarE with accum_out giving row sums for free;
no max subtraction (scores are O(10) for this model so exp cannot
overflow in f32). P is transposed 128x128 on TensorE (identity
matmul); transposes are emitted TWO score-chains late so the PE never
waits on the exp that produces their input. Score order is slot-0 key
chunks for all q-tiles first (slot-0 readback lands first), then kc in
{2,3} per q-tile so each tile's softmax reciprocal is ready before its
attention epilogue.
"""

import numpy as np

B, S, D = 4, 2048, 1024
SQ = S // 2  # queries/keys owned per core
NCORES = 8
DC = D // 128  # contraction chunks
EC = D // 128  # embed chunks
SC = S // 128  # key chunks, full batch
SCH = SQ // 128  # key chunks per half
QT = SQ // 128  # query tiles per core
KC = S // 512  # score column chunks, full batch
KCH = SQ // 512  # score column chunks per half
EJ = D // 512  # 512-wide embed column chunks

_cache = {}


def _build():
    import concourse.bass as bass
    import concourse.tile as tile
    from concourse import bacc, mybir
    from concourse.masks import make_identity

    f32 = mybir.dt.float32
    bf16 = mybir.dt.bfloat16
    Alu = mybir.AluOpType
    Act = mybir.ActivationFunctionType

    nc = bacc.Bacc(None, target_bir_lowering=False, debug=False)

    xqT_d = nc.declare_dram_parameter("xqT", [D, SQ], bf16, isOutput=False)
    wqE_d = nc.declare_dram_parameter("wqE", [EC, 128, DC, 128], bf16, isOutput=False)
    wkE_d = nc.declare_dram_parameter("wkE", [EC, 128, DC, 128], bf16, isOutput=False)
    wv_d = nc.declare_dram_parameter("wvT", [D, D], bf16, isOutput=False)
    bq_d = nc.declare_dram_parameter("bq", [D], f32, isOutput=False)
    out_d = nc.declare_dram_parameter("out", [SQ, D], f32, isOutput=True)

    # Pairwise K/V exchange staging; AllGather slot order = natural
    # batch order, identical on both pair members (uniform SPMD graph).
    kx_in = nc.dram_tensor("kx_in", [EC, 128, SQ], bf16)
    kx_out = nc.dram_tensor("kx_out", [2, EC, 128, SQ], bf16)
    vx_in = nc.dram_tensor("vx_in", [SCH, 128, D], bf16)
    vx_out = nc.dram_tensor("vx_out", [2, SCH, 128, D], bf16)

    groups = [[0, 1], [2, 3], [4, 5], [6, 7]]

    with tile.TileContext(nc) as tc:
        with tc.tile_pool(name="pers", bufs=1) as pers:
            qT_sb = pers.tile([128, EC, SQ], bf16, tag="qT")
            kT_sb = pers.tile([128, 2, EC, SQ], bf16, tag="kT")
            v_sb = pers.tile([128, 2, SCH, D], bf16, tag="v")
            ident = pers.tile([128, 128], bf16, tag="ident")
            make_identity(nc, ident)
            bq_sb = pers.tile([128, EC], f32, tag="bq")

            # PE warmup: dense dummy matmuls while the first input DMAs land,
            # so the HAM clock gate is already ramped when real work starts.
            warm_sb = pers.tile([128, 512], bf16, tag="warm")
            warm_dump = pers.tile([128, 512], f32, tag="warm_dump")
            nc.vector.memset(warm_sb, 0.0)
            with tc.tile_pool(name="warm_ps", bufs=1, space="PSUM") as warm_ps:
                wps = warm_ps.tile([128, 512], f32, tag="wps")
                NWARM = 10
                for i in range(NWARM):
                    nc.tensor.matmul(
                        wps,
                        lhsT=warm_sb[:, 0:128],
                        rhs=warm_sb,
                        start=(i == 0),
                        stop=(i == NWARM - 1),
                    )
                nc.vector.tensor_copy(out=warm_dump, in_=wps)

            bq_ap = bq_d.ap()
            nc.scalar.dma_start(
                out=bq_sb,
                in_=bass.AP(tensor=bq_ap.tensor, offset=0, ap=[[1, 128], [128, EC]]),
            )

            with (
                tc.tile_pool(name="ld", bufs=1) as ld,
                tc.tile_pool(name="stage", bufs=4) as stage,
                tc.tile_pool(name="proj_ps", bufs=4, space="PSUM") as proj_ps,
            ):
                xqT_sb = ld.tile([128, DC, SQ], bf16, tag="xqT")
                wk_sb = ld.tile([128, EC, DC, 128], bf16, tag="wk")
                wq_sb = ld.tile([128, EC, DC, 128], bf16, tag="wq")
                wv_sb = ld.tile([128, DC, D], bf16, tag="wv")

                # DMA priority: xqT gates every projection; wk e-chunks gate
                # the kT chains (and with them the K exchange); then wv, wq.
                for dc in range(DC):
                    r = slice(dc * 128, (dc + 1) * 128)
                    nc.sync.dma_start(out=xqT_sb[:, dc, :], in_=xqT_d[r, :])
                for ec in range(EC):
                    nc.sync.dma_start(out=wk_sb[:, ec, :, :], in_=wkE_d[ec])
                for dc in range(DC):
                    r = slice(dc * 128, (dc + 1) * 128)
                    nc.sync.dma_start(out=wv_sb[:, dc, :], in_=wv_d[r, :])
                for ec in range(EC):
                    nc.sync.dma_start(out=wq_sb[:, ec, :, :], in_=wqE_d[ec])

                # kT own half [e, sk_own] -> bf16 stage tiles -> DRAM.
                # One [128, SQ] stage tile per ec: fewer, larger stage DMAs
                # means fewer semaphore groups for the collective trigger.
                for ec in range(EC):
                    kst = stage.tile([128, SQ], bf16, tag="kst")
                    for kc in range(KCH):
                        csl = slice(kc * 512, (kc + 1) * 512)
                        ps = proj_ps.tile([128, 512], f32, tag="ps")
                        for dc in range(DC):
                            nc.tensor.matmul(
                                ps,
                                lhsT=wk_sb[:, ec, dc, :],
                                rhs=xqT_sb[:, dc, csl],
                                start=(dc == 0),
                                stop=(dc == DC - 1),
                            )
                        nc.vector.tensor_copy(out=kst[:, csl], in_=ps)
                    nc.scalar.dma_start(out=kx_in[ec], in_=kst)
                nc.gpsimd.collective_compute(
                    "AllGather",
                    Alu.bypass,
                    replica_groups=groups,
                    ins=[kx_in.ap().opt()],
                    outs=[kx_out.ap().opt()],
                )
                # Readbacks: one big strided DMA per slot on the sync queue
                # (idle after the input loads); slot 0 first — the kc-outer
                # score order consumes it first. gpsimd carries ONLY
                # collective triggers.
                kx_out_ap = kx_out.ap()
                for s in range(2):
                    nc.sync.dma_start(
                        out=kT_sb[:, s, :, :],
                        in_=bass.AP(
                            tensor=kx_out_ap.tensor,
                            offset=s * (EC * 128 * SQ),
                            ap=[[SQ, 128], [128 * SQ, EC], [1, SQ]],
                        ),
                    )

                # v own half [sk_own, e], same pattern.
                for sc in range(SCH):
                    vst = stage.tile([128, D], bf16, tag="vst")
                    for j in range(EJ):
                        jsl = slice(j * 512, (j + 1) * 512)
                        ps = proj_ps.tile([128, 512], f32, tag="ps")
                        for dc in range(DC):
                            nc.tensor.matmul(
                                ps,
                                lhsT=xqT_sb[:, dc, sc * 128 : (sc + 1) * 128],
                                rhs=wv_sb[:, dc, jsl],
                                start=(dc == 0),
                                stop=(dc == DC - 1),
                            )
                        nc.vector.tensor_copy(out=vst[:, jsl], in_=ps)
                    nc.scalar.dma_start(out=vx_in[sc], in_=vst)
                nc.gpsimd.collective_compute(
                    "AllGather",
                    Alu.bypass,
                    replica_groups=groups,
                    ins=[vx_in.ap().opt()],
                    outs=[vx_out.ap().opt()],
                )
                vx_out_ap = vx_out.ap()
                for s in range(2):
                    nc.sync.dma_start(
                        out=v_sb[:, s, :, :],
                        in_=bass.AP(
                            tensor=vx_out_ap.tensor,
                            offset=s * (SCH * 128 * D),
                            ap=[[D, 128], [128 * D, SCH], [1, D]],
                        ),
                    )

                # qT[e, sq] = sum_d wqT[d, e] * xqT[d, sq]  (+bq per-partition)
                # ec-OUTER: the first score chain depends on adds (ec0..7, j0)
                # — the 15th of 16 chains — so the scheduler cannot hoist it
                # ahead of meaningful qT cover while the K AllGather flies.
                for ec in range(EC):
                    for j in range(SQ // 512):
                        jsl = slice(j * 512, (j + 1) * 512)
                        ps = proj_ps.tile([128, 512], f32, tag="ps")
                        for dc in range(DC):
                            nc.tensor.matmul(
                                ps,
                                lhsT=wq_sb[:, ec, dc, :],
                                rhs=xqT_sb[:, dc, jsl],
                                start=(dc == 0),
                                stop=(dc == DC - 1),
                            )
                        nc.vector.tensor_scalar_add(
                            out=qT_sb[:, ec, jsl],
                            in0=ps,
                            scalar1=bq_sb[:, ec : ec + 1],
                        )

            with (
                tc.tile_pool(name="att", bufs=2) as att,
                tc.tile_pool(name="small", bufs=2) as small,
                tc.tile_pool(name="mm_ps", bufs=4, space="PSUM") as mm_ps,
                tc.tile_pool(name="tr_ps", bufs=3, space="PSUM") as tr_ps,
            ):
                inv_sqrt_d = float(1.0 / np.sqrt(D))
                P_list = [
                    att.tile([128, S], bf16, name=f"P{qt}", tag=f"P{qt}", bufs=1)
                    for qt in range(QT)
                ]
                PT_list = [
                    att.tile(
                        [128, SC, 128], bf16, name=f"PT{qt}", tag=f"PT{qt}", bufs=1
                    )
                    for qt in range(QT)
                ]
                den4_list = [
                    small.tile([128, KC], f32, name=f"den4{qt}", tag=f"den4{qt}", bufs=1)
                    for qt in range(QT)
                ]
                recip_list = [
                    small.tile(
                        [128, 1], f32, name=f"recip{qt}", tag=f"recip{qt}", bufs=1
                    )
                    for qt in range(QT)
                ]

                units = [(kc, qt) for kc in range(KCH) for qt in range(QT)]
                units += [(kc, qt) for qt in range(QT) for kc in range(KCH, KC)]

                def emit_transposes(kc, qt):
                    for j in range(kc * 4, kc * 4 + 4):
                        tp = tr_ps.tile([128, 128], bf16, tag="tr")
                        nc.tensor.transpose(
                            tp, P_list[qt][:, j * 128 : (j + 1) * 128], ident
                        )
                        nc.vector.tensor_copy(out=PT_list[qt][:, j, :], in_=tp)

                for i, (kc, qt) in enumerate(units):
                    csl = slice(kc * 512, (kc + 1) * 512)
                    qsl = slice(qt * 128, (qt + 1) * 128)
                    ps = mm_ps.tile([128, 512], f32, tag="mm")
                    for ec in range(EC):
                        nc.tensor.matmul(
                            ps,
                            lhsT=qT_sb[:, ec, qsl],
                            rhs=kT_sb[:, kc // KCH, ec, (kc % KCH) * 512 : (kc % KCH) * 512 + 512],
                            start=(ec == 0),
                            stop=(ec == EC - 1),
                        )
                    nc.scalar.activation(
                        out=P_list[qt][:, csl],
                        in_=ps,
                        func=Act.Exp,
                        scale=inv_sqrt_d,
                        accum_out=den4_list[qt][:, kc : kc + 1],
                    )
                    if i >= 2:
                        emit_transposes(*units[i - 2])
                    if kc == KC - 1:
                        den = small.tile([128, 1], f32, tag="den", bufs=4)
                        nc.vector.reduce_sum(
                            out=den, in_=den4_list[qt], axis=mybir.AxisListType.X
                        )
                        nc.vector.reciprocal(recip_list[qt], den)
                emit_transposes(*units[-2])
                emit_transposes(*units[-1])

                # pass 2: attn + scaled epilogue per q-tile. The psum tiles
                # come from the SAME ring as the score psums, pinning these
                # chains behind pass 1 in the PE stream (after the V
                # readback). Residual is added on the host.
                for qt in range(QT):
                    qsl = slice(qt * 128, (qt + 1) * 128)
                    PT_sb = PT_list[qt]
                    recip = recip_list[qt]
                    ot = att.tile([128, D], f32, tag="ot", bufs=3)
                    for j2 in range(EJ):
                        jsl = slice(j2 * 512, (j2 + 1) * 512)
                        pa = mm_ps.tile([128, 512], f32, tag="mm")
                        for j in range(SC):
                            nc.tensor.matmul(
                                pa,
                                lhsT=PT_sb[:, j, :],
                                rhs=v_sb[:, j // SCH, j % SCH, jsl],
                                start=(j == 0),
                                stop=(j == SC - 1),
                            )
                        nc.vector.tensor_scalar_mul(
                            out=ot[:, jsl], in0=pa, scalar1=recip
                        )
                        nc.scalar.dma_start(out=out_d[qsl, jsl], in_=ot[:, jsl])

    nc.compile()
    return nc


def _get_nc():
    if "nc" not in _cache:
        _cache["nc"] = _build()
    return _cache["nc"]


def kernel(embedded, Wq, bq, Wk, bk, Wv, bv):
    import ml_dtypes

    from concourse.bass_utils import run_bass_kernel_spmd

    bf16 = ml_dtypes.bfloat16
    x = np.ascontiguousarray(np.asarray(embedded, dtype=np.float32))
    Wq = np.asarray(Wq, dtype=np.float32)
    Wk = np.asarray(Wk, dtype=np.float32)
    Wv = np.asarray(Wv, dtype=np.float32)
    bq = np.ascontiguousarray(np.asarray(bq, dtype=np.float32))
    bk = np.ascontiguousarray(np.asarray(bk, dtype=np.float32))
    bv = np.ascontiguousarray(np.asarray(bv, dtype=np.float32))

    # e-chunk-major weight layouts: wE[ec, p, dc, j] = W.T[dc*128+p, ec*128+j]
    def echunk(wT):
        return np.ascontiguousarray(
            wT.reshape(DC, 128, EC, 128).transpose(2, 1, 0, 3)
        )

    wqT = np.ascontiguousarray(Wq.T).astype(bf16)
    wkT = np.ascontiguousarray(Wk.T).astype(bf16)
    wvT = np.ascontiguousarray(Wv.T).astype(bf16)
    wqE = echunk(wqT)
    wkE = echunk(wkT)
    xT = [np.ascontiguousarray(x[b].T).astype(bf16) for b in range(B)]

    in_maps = []
    for c in range(NCORES):
        b, h = c // 2, c % 2
        qs = slice(h * SQ, (h + 1) * SQ)
        in_maps.append(
            {
                "xqT": np.ascontiguousarray(xT[b][:, qs]),
                "wqE": wqE,
                "wkE": wkE,
                "wvT": wvT,
                "bq": bq,
            }
        )

    _cache["in_maps"] = in_maps
    nc = _get_nc()
    res = run_bass_kernel_spmd(nc, in_maps, core_ids=list(range(NCORES)))
    out = np.empty((B, S, D), dtype=np.float32)
    for c in range(NCORES):
        b, h = c // 2, c % 2
        out[b, h * SQ : (h + 1) * SQ, :] = res.results[c]["out"]
    # residual (+ V bias, which passes through the attention average)
    out += x + bv
    return out
